# revision 6
# baseline (speedup 1.0000x reference)
"""PALU low-rank Llama attention on 8 Trainium2 NeuronCores.

Tensor-parallel over the 8 PALU groups (1 group = 4 heads per core), with each
core computing a partial contribution to the full output (its group's slice of
the fused Wo contraction); the host sums the 8 partials.

v2: all-bf16 datapath (f32 PSUM accumulation), transpose-free Phase A, PSUM
bank staggering so per-chunk postludes overlap the next chunk's accumulation.

Per-core pipeline (one Bass/Tile kernel, SPMD over 8 cores with per-core
weight shards as inputs):
  A) projections in 256-token chunks: Q.T and the low-rank-reconstructed K.T
     are computed directly in (d, tokens) layout (head-dim on partitions) by
     using the weight tile as the stationary matmul operand; V is computed
     natural (tokens, f'). RoPE runs in the transposed layout with
     cross-partition rotate-half on the DVE and host-transposed cos/sin
     tables. No TensorE transposes anywhere in Phase A. PSUM: each chunk
     accumulates into 4 banks (klatT packed 2-per-bank, V 2-per-bank, Q 2x
     full), double-buffered across chunks so RoPE/recon of chunk i overlap
     the matmul accumulation of chunk i+1.
  B) attention per (batch, head): scores computed transposed S.T = K.T' Q
     (keys on partitions) in 512-query chunks, causal tiles only, mask added
     on diagonal 128x128 subtiles, exp on ScalarE straight out of PSUM into
     bf16, then O = P.T' V with two all-ones V columns producing the softmax
     denominators for free. O is normalized with per-partition reciprocal
     scalars and transposed (bf16, TensorE) into O.T buffers for stage C.
  C) fused output projection with the exact torch reshape/transpose
     semantics folded into strided access patterns on O.T.
"""

import numpy as np
import ml_dtypes

import concourse.bass as bass
import concourse.tile as tile
from concourse import bacc, mybir
from concourse.masks import make_identity

F32 = mybir.dt.float32
BF16 = mybir.dt.bfloat16
NPBF = ml_dtypes.bfloat16
EXP = mybir.ActivationFunctionType.Exp

B, S, HID = 2, 2048, 4096
NH, D = 32, 128
G, GS = 8, 4
RK, RV, FGD, GD = 256, 2048, 256, 512
NCORES = 8
CT = HID // 128        # 32 contraction tiles over hidden dim
NCH = S // 256         # 8 chunks of 256 tokens (phase A)
NQC = S // 512         # 4 q-chunks of 512 (phase B)
THETA = 10000.0

_NC_CACHE = {}


def _install_loud_cc_hook():
    """Surface exceptions thrown inside the neuronx_cc compile hook (the C++
    callback boundary otherwise swallows them into an opaque INTERNAL error)."""
    if _NC_CACHE.get("loud_hook"):
        return
    import traceback
    from concourse import bass2jax
    orig = bass2jax.neuronx_cc_hook

    def loud_hook(*a, **kw):
        try:
            return orig(*a, **kw)
        except BaseException:
            traceback.print_exc()
            raise

    bass2jax.neuronx_cc_hook = loud_hook
    _NC_CACHE["loud_hook"] = True


def _copy(eng_nc, dst, src):
    """Engine-agnostic copy: ScalarE uses activation-Copy, VectorE tensor_copy."""
    if hasattr(eng_nc, "tensor_copy"):
        eng_nc.tensor_copy(dst, src)
    else:
        eng_nc.copy(dst, src)


def _build_nc():
    """Build + compile the per-core Bass kernel (same NEFF for all cores)."""
    nc = bacc.Bacc(trn_type="TRN2", target_bir_lowering=False, debug=False)

    hid_d = nc.dram_tensor("hidT", [B, HID, S], BF16, kind="ExternalInput").ap()
    wq_d = nc.dram_tensor("wqT", [HID, GD], BF16, kind="ExternalInput").ap()
    wk_d = nc.dram_tensor("wkT", [HID, RK], BF16, kind="ExternalInput").ap()
    wv_d = nc.dram_tensor("wvT", [HID, FGD], BF16, kind="ExternalInput").ap()
    ug_d = nc.dram_tensor("ugT", [2, 128, GD], BF16, kind="ExternalInput").ap()
    cos_d = nc.dram_tensor("cosT", [B, 128, S], BF16, kind="ExternalInput").ap()
    sin_d = nc.dram_tensor("sinT", [B, 128, S], BF16, kind="ExternalInput").ap()
    msk_d = nc.dram_tensor("maskT", [B, S // 128, 128, 128], F32,
                           kind="ExternalInput").ap()
    wo_d = nc.dram_tensor("woT", [4, 2, 128, HID], BF16, kind="ExternalInput").ap()
    out_d = nc.dram_tensor("out", [B, S, HID], F32, kind="ExternalOutput").ap()

    # DRAM scratch (per-core internal)
    qT_s = nc.dram_tensor("qT_s", [B, GS, 128, S], BF16).ap()
    kT_s = nc.dram_tensor("kT_s", [B, GS, 128, S], BF16).ap()

    with tile.TileContext(nc) as tc:
        with tc.tile_pool(name="const", bufs=1) as pc:
            ident = pc.tile([128, 128], BF16)
            make_identity(nc, ident)

            # Persistent attention buffers (ot written in B, read in C;
            # v_t written by Phase A directly -- V never touches DRAM).
            potb = tc.tile_pool(name="otb", bufs=1).__enter__()
            pvt = tc.tile_pool(name="vtb", bufs=1).__enter__()
            ot = {}
            for b in range(B):
                for hl in range(GS):
                    for fp in range(2):
                        ot[(b, hl, fp)] = potb.tile(
                            [128, S], BF16, name=f"ot{b}{hl}{fp}",
                            tag=f"ot{b}{hl}{fp}")
            vt_tiles = {}
            for b in range(B):
                for kt in range(S // 128):
                    vt_tiles[(b, kt)] = pvt.tile(
                        [128, FGD + 2], BF16, name=f"v{b}_{kt}",
                        tag=f"v{b}_{kt}")

            # ================= Phase A: projections =================
            with tc.tile_pool(name="wts", bufs=1) as pw, \
                 tc.tile_pool(name="tabs", bufs=2) as pcs, \
                 tc.tile_pool(name="ht", bufs=8) as pht, \
                 tc.tile_pool(name="aev", bufs=2) as pev, \
                 tc.tile_pool(name="rope", bufs=4) as prope, \
                 tc.tile_pool(name="apsum", bufs=1, space="PSUM") as pA:

                # Weight tiles; DMAs are emitted interleaved with the first
                # chunk's hidden tiles (below) so the first matmuls start
                # within ~2us instead of waiting for the full weight load.
                wq_sb, wk_sb, wv_sb = [], [], []
                for ct in range(CT):
                    wk_sb.append(pw.tile([128, RK], BF16, tag=f"wk{ct}", name=f"wk{ct}"))
                    wv_sb.append(pw.tile([128, FGD], BF16, tag=f"wv{ct}", name=f"wv{ct}"))
                    wq_sb.append(pw.tile([128, GD], BF16, tag=f"wq{ct}", name=f"wq{ct}"))
                ug_sb = []
                for rt in range(2):
                    ug_sb.append(pw.tile([128, GD], BF16, tag=f"ug{rt}", name=f"ug{rt}"))

                def rope_spill(xp, dst, b, h, c0, cosb, sinb):
                    """xp: [128 d, 256 tok] (PSUM f32). Apply RoPE in the
                    transposed layout and spill bf16 to dst[b, h][:, c0:]."""
                    tmp = prope.tile([128, 256], F32, tag="tmp")
                    rot = prope.tile([128, 256], F32, tag="rot")
                    cs = cosb[:, c0:c0 + 256]
                    sn = sinb[:, c0:c0 + 256]
                    nc.vector.tensor_mul(tmp[:], xp, cs)
                    nc.vector.tensor_mul(rot[0:64, :], xp[64:128, :],
                                         sn[0:64, :])
                    nc.vector.tensor_mul(rot[64:128, :], xp[0:64, :],
                                         sn[64:128, :])
                    ev = prope.tile([128, 256], BF16, tag="ev")
                    nc.vector.tensor_add(ev[:], tmp[:], rot[:])
                    nc.sync.dma_start(dst[b, h, :, c0:c0 + 256], ev[:])

                for b in range(B):
                    cosb = pcs.tile([128, S], BF16, tag="cosb")
                    sinb = pcs.tile([128, S], BF16, tag="sinb")
                    if b > 0:
                        nc.sync.dma_start(cosb[:], cos_d[b])
                        nc.sync.dma_start(sinb[:], sin_d[b])

                    for ch in range(NCH):
                        c0 = ch * 256
                        klp = pA.tile([128, 512], F32, tag="kl", bufs=1)
                        vnp = pA.tile([128, 512], F32, tag="v", bufs=1)
                        qp = [pA.tile([128, 512], F32, tag="q", bufs=4,
                                      name=f"qp{i}") for i in range(2)]
                        for ct in range(CT):
                            first = (b == 0 and ch == 0)
                            if first:
                                nc.sync.dma_start(
                                    wk_sb[ct][:],
                                    wk_d[ct * 128:(ct + 1) * 128, :])
                                nc.sync.dma_start(
                                    wv_sb[ct][:],
                                    wv_d[ct * 128:(ct + 1) * 128, :])
                            t = pht.tile([128, 256], BF16, tag="ht")
                            nc.sync.dma_start(
                                t[:], hid_d[b, ct * 128:(ct + 1) * 128,
                                            c0:c0 + 256])
                            if first:
                                nc.sync.dma_start(
                                    wq_sb[ct][:],
                                    wq_d[ct * 128:(ct + 1) * 128, :])
                            st, sp = (ct == 0), (ct == CT - 1)
                            # Q.T: stationary = wq head block, moving = hidden
                            for h in range(GS):
                                nc.tensor.matmul(
                                    qp[h // 2][:, (h % 2) * 256:
                                               (h % 2) * 256 + 256],
                                    wq_sb[ct][:, h * 128:(h + 1) * 128],
                                    t[:], start=st, stop=sp)
                            # K latent transposed: (latent, tok)
                            for rt in range(2):
                                nc.tensor.matmul(
                                    klp[:, rt * 256:(rt + 1) * 256],
                                    wk_sb[ct][:, rt * 128:(rt + 1) * 128],
                                    t[:], start=st, stop=sp)
                            # V natural: (tok, f')
                            for qb in range(2):
                                nc.tensor.matmul(
                                    vnp[:, qb * 256:(qb + 1) * 256],
                                    t[:, qb * 128:(qb + 1) * 128],
                                    wv_sb[ct][:], start=st, stop=sp)

                        if b == 0 and ch == 0:
                            for rt in range(2):
                                nc.sync.dma_start(ug_sb[rt][:], ug_d[rt])
                            nc.sync.dma_start(cosb[:], cos_d[b])
                            nc.sync.dma_start(sinb[:], sin_d[b])

                        # ---- chunk postlude (overlaps next chunk's accum) ----
                        klT = pev.tile([128, 512], BF16, tag="klT")
                        nc.scalar.copy(klT[:], klp[:])
                        for qb in range(2):
                            v_t = vt_tiles[(b, 2 * ch + qb)]
                            nc.vector.tensor_copy(
                                v_t[:, :FGD], vnp[:, qb * 256:(qb + 1) * 256])
                            nc.vector.memset(v_t[:, FGD:FGD + 2], 1.0)
                        for hp in range(2):
                            kp = pA.tile([128, 512], F32, tag="kp", bufs=2)
                            for hi in range(2):
                                h = hp * 2 + hi
                                ksl = kp[:, hi * 256:(hi + 1) * 256]
                                for rt in range(2):
                                    nc.tensor.matmul(
                                        ksl,
                                        ug_sb[rt][:, h * 128:(h + 1) * 128],
                                        klT[:, rt * 256:(rt + 1) * 256],
                                        start=(rt == 0), stop=(rt == 1))
                                rope_spill(ksl, kT_s, b, h, c0, cosb, sinb)
                                rope_spill(qp[h // 2][:, (h % 2) * 256:
                                                      (h % 2) * 256 + 256],
                                           qT_s, b, h, c0, cosb, sinb)

            # ================= Phase B: attention =================
            if True:
                with tc.tile_pool(name="mkb", bufs=16) as pmk, \
                     tc.tile_pool(name="ktb", bufs=2) as pkt, \
                     tc.tile_pool(name="qtb", bufs=2) as pqt, \
                     tc.tile_pool(name="ptb", bufs=4) as ppt, \
                     tc.tile_pool(name="bsm", bufs=4) as pbs, \
                     tc.tile_pool(name="wob", bufs=1) as pwo, \
                     tc.tile_pool(name="bpsum", bufs=1, space="PSUM") as bps:

                    wo_t = {}
                    for b in range(B):
                        vt = [vt_tiles[(b, kt)] for kt in range(S // 128)]
                        mk = []
                        for qt in range(S // 128):
                            m_t = pmk.tile([128, 128], F32,
                                           name=f"m{b}_{qt}", tag="mk")
                            nc.sync.dma_start(m_t[:], msk_d[b, qt])
                            mk.append(m_t)

                        for hl in range(GS):
                            kt_sb = pkt.tile([128, S], BF16, tag="kt")
                            nc.sync.dma_start(kt_sb[:], kT_s[b, hl])
                            for qc in range(NQC):
                                qt_sb = pqt.tile([128, 512], BF16, tag="qt")
                                nc.sync.dma_start(
                                    qt_sb[:],
                                    qT_s[b, hl, :, qc * 512:(qc + 1) * 512])
                                o_ps = [bps.tile([128, 512], F32, tag="ob",
                                                 bufs=4, name=f"ob{i}")
                                        for i in range(4)]
                                nkt = 4 * qc + 4
                                for kt in range(nkt):
                                    sc = bps.tile([128, 512], F32, tag="sc",
                                                  bufs=2)
                                    nc.tensor.matmul(
                                        sc[:],
                                        kt_sb[:, kt * 128:(kt + 1) * 128],
                                        qt_sb[:], start=True, stop=True)
                                    if kt >= 4 * qc:
                                        sub = kt - 4 * qc
                                        sl = sc[:, sub * 128:(sub + 1) * 128]
                                        nc.vector.tensor_add(sl, sl, mk[kt][:])
                                    pt = ppt.tile([128, 512], BF16, tag="pt")
                                    nc.scalar.activation(pt[:], sc[:], EXP)
                                    for sub in range(4):
                                        if kt <= 4 * qc + sub:
                                            nc.tensor.matmul(
                                                o_ps[sub][:, :FGD + 2],
                                                pt[:, sub * 128:
                                                   (sub + 1) * 128],
                                                vt[kt][:],
                                                start=(kt == 0),
                                                stop=(kt == 4 * qc + sub))
                                recip = pbs.tile([128, 4], F32, tag="rc")
                                for sub in range(4):
                                    nc.vector.reciprocal(
                                        recip[:, sub:sub + 1],
                                        o_ps[sub][:, FGD:FGD + 1])
                                for sub in range(4):
                                    o_sb = pbs.tile([128, FGD], BF16,
                                                    tag="osb")
                                    nc.vector.tensor_scalar_mul(
                                        o_sb[:], o_ps[sub][:, :FGD],
                                        recip[:, sub:sub + 1])
                                    tp2 = bps.tile([128, 256], BF16, tag="tp2",
                                                   bufs=2)
                                    for fp in range(2):
                                        nc.tensor.transpose(
                                            tp2[:, fp * 128:(fp + 1) * 128],
                                            o_sb[:, fp * 128:(fp + 1) * 128],
                                            ident[:])
                                    col = qc * 512 + sub * 128
                                    eng = nc.scalar if sub % 2 else nc.vector
                                    for fp in range(2):
                                        _copy(eng,
                                              ot[(b, hl, fp)][:, col:col + 128],
                                              tp2[:, fp * 128:(fp + 1) * 128])

                        if b == 0:
                            # prefetch Wo while b=1 attention runs
                            for half in range(2):
                                wo_t[half] = pwo.tile(
                                    [128, 8 * 2048], BF16,
                                    name=f"wo{half}", tag=f"wo{half}")
                                for j in range(4):
                                    for fp in range(2):
                                        nc.sync.dma_start(
                                            wo_t[half][:, (j * 2 + fp) * 2048:
                                                 (j * 2 + fp + 1) * 2048],
                                            wo_d[j, fp, :,
                                                 half * 2048:(half + 1) * 2048])

                # ================= Phase C: output projection =================
                with tc.tile_pool(name="cev", bufs=4) as pcev, \
                     tc.tile_pool(name="cpsum", bufs=1, space="PSUM") as cps:
                    for half in range(2):
                        for b in range(B):
                            for tt in range(S // 128):
                                hl = tt // 4
                                for mci in range(4):
                                    mc = half * 4 + mci
                                    ops = cps.tile([128, 512], F32, tag="oc",
                                                   bufs=4)
                                    for j in range(4):
                                        for fp in range(2):
                                            otr = ot[(b, hl, fp)][:].rearrange(
                                                "p (x s) -> p x s", s=4)
                                            lhsT = otr[:, (tt % 4) * 128:
                                                       (tt % 4 + 1) * 128, j]
                                            rhs = wo_t[half][
                                                :, (j * 2 + fp) * 2048
                                                + mci * 512:
                                                (j * 2 + fp) * 2048
                                                + (mci + 1) * 512]
                                            nc.tensor.matmul(
                                                ops[:], lhsT, rhs,
                                                start=(j == 0 and fp == 0),
                                                stop=(j == 3 and fp == 1))
                                    ev = pcev.tile([128, 512], F32, tag="cev")
                                    eng = nc.scalar if (tt + mci) % 2 else nc.vector
                                    _copy(eng, ev[:], ops[:])
                                    nc.sync.dma_start(
                                        out_d[b, tt * 128:(tt + 1) * 128,
                                              mc * 512:(mc + 1) * 512],
                                        ev[:])

            pvt.__exit__(None, None, None)
            potb.__exit__(None, None, None)

    nc.compile()
    return nc


def _host_prep(inputs):
    """Slice/transposes per core; returns (in_maps, fallback_needed)."""
    hs = np.ascontiguousarray(inputs["hidden_states"], dtype=np.float32)
    mask = np.ascontiguousarray(inputs["attention_mask"], dtype=np.float32)
    pos = np.asarray(inputs["position_ids"])
    Wq = np.asarray(inputs["Wq"], dtype=np.float32)
    WVT = np.asarray(inputs["WVT"], dtype=np.float32)
    U = np.asarray(inputs["U"], dtype=np.float32)
    Wv = np.asarray(inputs["Wv"], dtype=np.float32)
    Wo = np.asarray(inputs["Wo"], dtype=np.float32)

    # Verify causal-family mask: strictly-lower 128-blocks all zero,
    # strictly-upper all <= -1e8 (else fall back to numpy reference).
    nt = S // 128
    mb = mask.reshape(B, nt, 128, nt, 128).transpose(0, 1, 3, 2, 4)
    lower_ok = True
    for b in range(B):
        for i in range(nt):
            for k in range(nt):
                blk = mb[b, i, k]
                if k < i and not np.all(blk == 0.0):
                    lower_ok = False
                if k > i and not np.all(blk <= -1e8):
                    lower_ok = False
    if not lower_ok:
        return None, True

    hidT = np.ascontiguousarray(hs.transpose(0, 2, 1)).astype(NPBF)
    hidT = np.ascontiguousarray(hidT.transpose(0, 2, 1))  # (B, HID, S) bf16

    # RoPE tables, transposed (d on partitions); sin sign-folded for the
    # transposed rotate-half.
    inv = 1.0 / (THETA ** (np.arange(0, D, 2, dtype=np.float32) / D))
    fr = pos.astype(np.float32)[..., None] * inv                # (B, S, 64)
    emb = np.concatenate([fr, fr], axis=-1)                     # (B, S, 128)
    cosT = np.ascontiguousarray(
        np.cos(emb).astype(np.float32).transpose(0, 2, 1)).astype(NPBF)
    sinT = np.sin(emb).astype(np.float32).transpose(0, 2, 1)    # (B, 128, S)
    sinTf = np.concatenate([-sinT[:, :64], sinT[:, 64:]], axis=1)
    sinTf = np.ascontiguousarray(sinTf).astype(NPBF)

    # Transposed diagonal mask tiles (k, q)
    maskT = np.ascontiguousarray(
        np.stack([np.stack([mask[b, 0, t * 128:(t + 1) * 128,
                                 t * 128:(t + 1) * 128].T
                            for t in range(nt)]) for b in range(B)]))

    scale = np.float32(1.0 / np.sqrt(D))
    in_maps = []
    for g in range(NCORES):
        wqT = np.ascontiguousarray(
            Wq[g * GD:(g + 1) * GD, :].T * scale).astype(NPBF)
        wkT = np.ascontiguousarray(WVT[g * RK:(g + 1) * RK, :].T).astype(NPBF)
        wvT = np.ascontiguousarray(Wv[g * RK:(g + 1) * RK, :].T).astype(NPBF)
        Ug = U[:, g * RK:(g + 1) * RK]                           # (GD, RK)
        ugT = np.stack([np.ascontiguousarray(Ug[:, rt * 128:(rt + 1) * 128].T)
                        for rt in range(2)]).astype(NPBF)        # (2,128,GD)
        woT = np.empty((4, 2, 128, HID), np.float32)
        for j in range(4):
            base = j * 2048 + g * FGD
            blk = Wo[:, base:base + FGD].T                       # (256, 4096)
            woT[j, 0] = blk[:128]
            woT[j, 1] = blk[128:]
        in_maps.append(dict(hidT=hidT, wqT=wqT, wkT=wkT, wvT=wvT, ugT=ugT,
                            cosT=cosT, sinT=sinTf, maskT=maskT,
                            woT=woT.astype(NPBF)))
    return in_maps, False


def _numpy_fallback(inputs):
    hs = np.asarray(inputs["hidden_states"], np.float32)
    mask = np.asarray(inputs["attention_mask"], np.float32)
    pos = np.asarray(inputs["position_ids"])
    Wq, WVT, U, Wv, Wo = (np.asarray(inputs[k], np.float32)
                          for k in ["Wq", "WVT", "U", "Wv", "Wo"])
    b, q = hs.shape[:2]
    qs = (hs @ Wq.T).reshape(b, q, NH, D).transpose(0, 2, 1, 3)
    klat = (hs @ WVT.T).reshape(b, q, G, RK).transpose(0, 2, 1, 3)
    vlat = (hs @ Wv.T).reshape(b, q, G, FGD).transpose(0, 2, 1, 3)
    Ugr = U.reshape(GD, G, RK)
    keys = np.einsum("bgsr,dgr->bgsd", klat, Ugr)
    keys = keys.transpose(0, 2, 1, 3).reshape(b, q, NH, D).transpose(0, 2, 1, 3)
    inv = 1.0 / (THETA ** (np.arange(0, D, 2, dtype=np.float32) / D))
    fr = pos.astype(np.float32)[..., None] * inv
    emb = np.concatenate([fr, fr], -1)
    cos, sin = np.cos(emb)[:, None], np.sin(emb)[:, None]

    def rot(x):
        return np.concatenate([-x[..., D // 2:], x[..., :D // 2]], -1)
    qs = qs * cos + rot(qs) * sin
    keys = keys * cos + rot(keys) * sin
    att = np.einsum("bhqd,bhkd->bhqk", qs, keys) / np.sqrt(D).astype(np.float32)
    att = att + mask
    att = att - att.max(-1, keepdims=True)
    att = np.exp(att)
    att = att / att.sum(-1, keepdims=True)
    aw = att.reshape(b, G, q * GS, q)
    o = np.einsum("bgik,bgkf->bgif", aw.astype(np.float32),
                  vlat.astype(np.float32))
    o = o.transpose(0, 2, 1, 3).reshape(b, q, 8192)
    return (o @ Wo.T).astype(np.float32)


def _make_timing_fn(nc):
    """Build the sharded jit callable for this Bass module.

    Mirrors bass2jax.run_bass_via_pjrt's multi-core path; returns
    (fn, in_names, out_avals, sharding)."""
    import jax
    from jax.sharding import Mesh, NamedSharding, PartitionSpec
    from jax.experimental.shard_map import shard_map
    from concourse import bass2jax, mybir as _mb

    bass2jax.install_neuronx_cc_hook()

    part_name = (nc.partition_id_tensor.name
                 if nc.partition_id_tensor is not None else None)
    in_names, out_names, out_avals = [], [], []
    for alloc in nc.m.functions[0].allocations:
        if not isinstance(alloc, _mb.MemoryLocationSet):
            continue
        name = alloc.memorylocations[0].name
        if alloc.kind == "ExternalInput":
            if name != part_name:
                in_names.append(name)
        elif alloc.kind == "ExternalOutput":
            out_names.append(name)
            out_avals.append(jax.core.ShapedArray(
                tuple(alloc.tensor_shape), _mb.dt.np(alloc.dtype)))
    n_params = len(in_names)
    all_names = in_names + out_names
    if part_name is not None:
        all_names = all_names + [part_name]

    def _body(*args):
        operands = list(args)
        if part_name is not None:
            operands.append(bass2jax.partition_id_tensor())
        outs = bass2jax._bass_exec_p.bind(
            *operands,
            out_avals=tuple(out_avals),
            in_names=tuple(all_names),
            out_names=tuple(out_names),
            lowering_input_output_aliases=(),
            sim_require_finite=True,
            sim_require_nnan=True,
            nc=nc,
        )
        return tuple(outs)

    devices = jax.devices()[:NCORES]
    mesh = Mesh(np.asarray(devices), ("core",))
    spec = PartitionSpec("core")
    n_outs = len(out_names)
    fn = jax.jit(
        shard_map(_body, mesh=mesh, in_specs=(spec,) * (n_params + n_outs),
                  out_specs=(spec,) * n_outs, check_rep=False),
        keep_unused=True,
    )
    return fn, in_names, out_names, out_avals, NamedSharding(mesh, spec)


def _run_spmd(nc, in_maps, time_iters=0):
    """Execute the SPMD kernel on the first NCORES neuron devices via PJRT.

    Keeps inputs device-resident so repeated executions can be wall-clocked.
    Returns (results_per_core, exec_ns_best or None)."""
    import time as _time

    import jax

    if "timing_fn" not in _NC_CACHE:
        _NC_CACHE["timing_fn"] = _make_timing_fn(nc)
    fn, in_names, out_names, out_avals, sharding = _NC_CACHE["timing_fn"]
    dev_in = [
        jax.device_put(
            np.concatenate([np.asarray(m[name]) for m in in_maps], axis=0),
            sharding)
        for name in in_names
    ]
    dev_zero = [
        jax.device_put(
            np.zeros((NCORES * a.shape[0], *a.shape[1:]), a.dtype), sharding)
        for a in out_avals
    ]
    out = jax.block_until_ready(fn(*dev_in, *dev_zero))

    exec_ns = None
    if time_iters > 0:
        times = []
        for _ in range(time_iters):
            t0 = _time.perf_counter()
            r = jax.block_until_ready(fn(*dev_in, *dev_zero))
            times.append(_time.perf_counter() - t0)
        del r
        exec_ns = int(min(times) * 1e9)
        _NC_CACHE["bench_times"] = times

    results = []
    for c in range(NCORES):
        results.append({
            name: np.asarray(out[i]).reshape(NCORES, *out_avals[i].shape)[c]
            for i, name in enumerate(out_names)
        })
    return results, exec_ns


def kernel(**inputs):
    import os

    in_maps, fallback = _host_prep(inputs)
    if fallback:
        return _numpy_fallback(inputs)

    _install_loud_cc_hook()
    if "nc" not in _NC_CACHE:
        _NC_CACHE["nc"] = _build_nc()
    nc = _NC_CACHE["nc"]

    iters = int(os.environ.get("TRN_KERNEL_TIME_ITERS", "0"))
    results, exec_ns = _run_spmd(nc, in_maps, time_iters=iters)
    _NC_CACHE["last_exec_ns"] = exec_ns

    acc = np.zeros((B, S, HID), np.float64)
    for r in results:
        acc += r["out"].astype(np.float64)
    return acc.astype(np.float32)


# revision 12
# speedup vs baseline: 1.1815x; 1.1815x over previous
"""PALU low-rank Llama attention on 8 Trainium2 NeuronCores.

Tensor-parallel over the 8 PALU groups (1 group = 4 heads per core), with each
core computing a partial contribution to the full output (its group's slice of
the fused Wo contraction); the host sums the 8 partials.

v2: all-bf16 datapath (f32 PSUM accumulation), transpose-free Phase A, PSUM
bank staggering so per-chunk postludes overlap the next chunk's accumulation.

Per-core pipeline (one Bass/Tile kernel, SPMD over 8 cores with per-core
weight shards as inputs):
  A) projections in 256-token chunks: Q.T and the low-rank-reconstructed K.T
     are computed directly in (d, tokens) layout (head-dim on partitions) by
     using the weight tile as the stationary matmul operand; V is computed
     natural (tokens, f'). RoPE runs in the transposed layout with
     cross-partition rotate-half on the DVE and host-transposed cos/sin
     tables. No TensorE transposes anywhere in Phase A. PSUM: each chunk
     accumulates into 4 banks (klatT packed 2-per-bank, V 2-per-bank, Q 2x
     full), double-buffered across chunks so RoPE/recon of chunk i overlap
     the matmul accumulation of chunk i+1.
  B) attention per (batch, head): scores computed transposed S.T = K.T' Q
     (keys on partitions) in 512-query chunks, causal tiles only, mask added
     on diagonal 128x128 subtiles, exp on ScalarE straight out of PSUM into
     bf16, then O = P.T' V with two all-ones V columns producing the softmax
     denominators for free. O is normalized with per-partition reciprocal
     scalars and transposed (bf16, TensorE) into O.T buffers for stage C.
  C) fused output projection with the exact torch reshape/transpose
     semantics folded into strided access patterns on O.T.
"""

import numpy as np
import ml_dtypes

import concourse.bass as bass
import concourse.tile as tile
from concourse import bacc, mybir
from concourse.masks import make_identity

F32 = mybir.dt.float32
BF16 = mybir.dt.bfloat16
NPBF = ml_dtypes.bfloat16
EXP = mybir.ActivationFunctionType.Exp

B, S, HID = 2, 2048, 4096
NH, D = 32, 128
G, GS = 8, 4
RK, RV, FGD, GD = 256, 2048, 256, 512
NCORES = 8
CT = HID // 128        # 32 contraction tiles over hidden dim
NCH = S // 256         # 8 chunks of 256 tokens (phase A)
NQC = S // 512         # 4 q-chunks of 512 (phase B)
THETA = 10000.0

_NC_CACHE = {}


def _install_loud_cc_hook():
    """Surface exceptions thrown inside the neuronx_cc compile hook (the C++
    callback boundary otherwise swallows them into an opaque INTERNAL error)."""
    if _NC_CACHE.get("loud_hook"):
        return
    import traceback
    from concourse import bass2jax
    orig = bass2jax.neuronx_cc_hook

    def loud_hook(*a, **kw):
        try:
            return orig(*a, **kw)
        except BaseException:
            traceback.print_exc()
            raise

    bass2jax.neuronx_cc_hook = loud_hook
    _NC_CACHE["loud_hook"] = True


def _copy(eng_nc, dst, src):
    """Engine-agnostic copy: ScalarE uses activation-Copy, VectorE tensor_copy."""
    if hasattr(eng_nc, "tensor_copy"):
        eng_nc.tensor_copy(dst, src)
    else:
        eng_nc.copy(dst, src)


def _build_nc():
    """Build + compile the per-core Bass kernel (same NEFF for all cores)."""
    nc = bacc.Bacc(trn_type="TRN2", target_bir_lowering=False, debug=False)

    hid_d = nc.dram_tensor("hidT", [B, HID, S], BF16, kind="ExternalInput").ap()
    wq_d = nc.dram_tensor("wqT", [HID, GD], BF16, kind="ExternalInput").ap()
    wk_d = nc.dram_tensor("wkT", [HID, RK], BF16, kind="ExternalInput").ap()
    wv_d = nc.dram_tensor("wvT", [HID, FGD], BF16, kind="ExternalInput").ap()
    ug_d = nc.dram_tensor("ugT", [2, 128, GD], BF16, kind="ExternalInput").ap()
    cos_d = nc.dram_tensor("cosT", [B, 128, S], BF16, kind="ExternalInput").ap()
    sin_d = nc.dram_tensor("sinT", [B, 128, S], BF16, kind="ExternalInput").ap()
    msk_d = nc.dram_tensor("maskT", [B, S // 128, 128, 128], F32,
                           kind="ExternalInput").ap()
    wo_d = nc.dram_tensor("woT", [4, 2, 128, HID], BF16, kind="ExternalInput").ap()
    out_d = nc.dram_tensor("out", [B, S, HID], F32, kind="ExternalOutput").ap()

    # DRAM scratch (per-core internal)
    qT_s = nc.dram_tensor("qT_s", [B, GS, 128, S], BF16).ap()
    kT_s = nc.dram_tensor("kT_s", [B, GS, 128, S], BF16).ap()

    with tile.TileContext(nc) as tc:
        with tc.tile_pool(name="const", bufs=1) as pc:
            ident = pc.tile([128, 128], BF16)
            make_identity(nc, ident)

            # Persistent attention buffers (ot written in B, read in C;
            # v_t written by Phase A directly -- V never touches DRAM).
            _potb_cm = tc.tile_pool(name="otb", bufs=1)
            _pvt_cm = tc.tile_pool(name="vtb", bufs=1)
            potb = _potb_cm.__enter__()
            pvt = _pvt_cm.__enter__()
            ot = {}
            for b in range(B):
                for hl in range(GS):
                    for fp in range(2):
                        ot[(b, hl, fp)] = potb.tile(
                            [128, S], BF16, name=f"ot{b}{hl}{fp}",
                            tag=f"ot{b}{hl}{fp}")
            vt_tiles = {}
            for b in range(B):
                for kt in range(S // 128):
                    vt_tiles[(b, kt)] = pvt.tile(
                        [128, FGD + 2], BF16, name=f"v{b}_{kt}",
                        tag=f"v{b}_{kt}")

            # ================= Phase A: projections =================
            with tc.tile_pool(name="wts", bufs=1) as pw, \
                 tc.tile_pool(name="tabs", bufs=2) as pcs, \
                 tc.tile_pool(name="ht", bufs=8) as pht, \
                 tc.tile_pool(name="aev", bufs=2) as pev, \
                 tc.tile_pool(name="rope", bufs=4) as prope, \
                 tc.tile_pool(name="apsum", bufs=1, space="PSUM") as pA:

                # Weight tiles; DMAs are emitted interleaved with the first
                # chunk's hidden tiles (below) so the first matmuls start
                # within ~2us instead of waiting for the full weight load.
                wq_sb, wk_sb, wv_sb = [], [], []
                for ct in range(CT):
                    wk_sb.append(pw.tile([128, RK], BF16, tag=f"wk{ct}", name=f"wk{ct}"))
                    wv_sb.append(pw.tile([128, FGD], BF16, tag=f"wv{ct}", name=f"wv{ct}"))
                    wq_sb.append(pw.tile([128, GD], BF16, tag=f"wq{ct}", name=f"wq{ct}"))
                ug_sb = []
                for rt in range(2):
                    ug_sb.append(pw.tile([128, GD], BF16, tag=f"ug{rt}", name=f"ug{rt}"))

                def rope_spill(xp, dst, b, h, c0, cosb, sinb):
                    """xp: [128 d, 256 tok] (PSUM f32). Apply RoPE in the
                    transposed layout and spill bf16 to dst[b, h][:, c0:]."""
                    tmp = prope.tile([128, 256], F32, tag="tmp")
                    rot = prope.tile([128, 256], F32, tag="rot")
                    cs = cosb[:, c0:c0 + 256]
                    sn = sinb[:, c0:c0 + 256]
                    nc.vector.tensor_mul(tmp[:], xp, cs)
                    nc.vector.tensor_mul(rot[0:64, :], xp[64:128, :],
                                         sn[0:64, :])
                    nc.vector.tensor_mul(rot[64:128, :], xp[0:64, :],
                                         sn[64:128, :])
                    ev = prope.tile([128, 256], BF16, tag="ev")
                    nc.vector.tensor_add(ev[:], tmp[:], rot[:])
                    nc.sync.dma_start(dst[b, h, :, c0:c0 + 256], ev[:])

                for b in range(B):
                    cosb = pcs.tile([128, S], BF16, tag="cosb")
                    sinb = pcs.tile([128, S], BF16, tag="sinb")
                    if b > 0:
                        nc.gpsimd.dma_start(cosb[:], cos_d[b])
                        nc.gpsimd.dma_start(sinb[:], sin_d[b])

                    for ch in range(NCH):
                        c0 = ch * 256
                        klp = pA.tile([128, 512], F32, tag="kl", bufs=1)
                        vnp = pA.tile([128, 512], F32, tag="v", bufs=1)
                        qp = [pA.tile([128, 512], F32, tag="q", bufs=4,
                                      name=f"qp{i}") for i in range(2)]
                        for ct in range(CT):
                            first = (b == 0 and ch == 0)
                            if first:
                                nc.gpsimd.dma_start(
                                    wk_sb[ct][:],
                                    wk_d[ct * 128:(ct + 1) * 128, :])
                                nc.gpsimd.dma_start(
                                    wv_sb[ct][:],
                                    wv_d[ct * 128:(ct + 1) * 128, :])
                            t = pht.tile([128, 256], BF16, tag="ht")
                            nc.sync.dma_start(
                                t[:], hid_d[b, ct * 128:(ct + 1) * 128,
                                            c0:c0 + 256])
                            if first:
                                nc.gpsimd.dma_start(
                                    wq_sb[ct][:],
                                    wq_d[ct * 128:(ct + 1) * 128, :])
                            st, sp = (ct == 0), (ct == CT - 1)
                            # Two accumulation groups share each PSUM
                            # bank: only the bank's first matmul may carry
                            # start (it zeroes the whole 2KB zero-region)
                            # and only its last carries stop.
                            # Q.T: stationary = wq head block, moving = hidden
                            for h in range(GS):
                                nc.tensor.matmul(
                                    qp[h // 2][:, (h % 2) * 256:
                                               (h % 2) * 256 + 256],
                                    wq_sb[ct][:, h * 128:(h + 1) * 128],
                                    t[:], start=st and h % 2 == 0,
                                    stop=sp and h % 2 == 1)
                            # K latent transposed: (latent, tok)
                            for rt in range(2):
                                nc.tensor.matmul(
                                    klp[:, rt * 256:(rt + 1) * 256],
                                    wk_sb[ct][:, rt * 128:(rt + 1) * 128],
                                    t[:], start=st and rt == 0,
                                    stop=sp and rt == 1)
                            # V natural: (tok, f')
                            for qb in range(2):
                                nc.tensor.matmul(
                                    vnp[:, qb * 256:(qb + 1) * 256],
                                    t[:, qb * 128:(qb + 1) * 128],
                                    wv_sb[ct][:], start=st and qb == 0,
                                    stop=sp and qb == 1)

                        if b == 0 and ch == 0:
                            for rt in range(2):
                                nc.gpsimd.dma_start(ug_sb[rt][:], ug_d[rt])
                            nc.gpsimd.dma_start(cosb[:], cos_d[b])
                            nc.gpsimd.dma_start(sinb[:], sin_d[b])

                        # ---- chunk postlude (overlaps next chunk's accum) ----
                        klT = pev.tile([128, 512], BF16, tag="klT")
                        nc.scalar.copy(klT[:], klp[:])
                        for qb in range(2):
                            v_t = vt_tiles[(b, 2 * ch + qb)]
                            nc.vector.tensor_copy(
                                v_t[:, :FGD], vnp[:, qb * 256:(qb + 1) * 256])
                            nc.vector.memset(v_t[:, FGD:FGD + 2], 1.0)
                        for hp in range(2):
                            kp = pA.tile([128, 512], F32, tag="kp", bufs=2)
                            for hi in range(2):
                                h = hp * 2 + hi
                                for rt in range(2):
                                    nc.tensor.matmul(
                                        kp[:, hi * 256:(hi + 1) * 256],
                                        ug_sb[rt][:, h * 128:(h + 1) * 128],
                                        klT[:, rt * 256:(rt + 1) * 256],
                                        start=(hi == 0 and rt == 0),
                                        stop=(hi == 1 and rt == 1))
                            for hi in range(2):
                                h = hp * 2 + hi
                                rope_spill(kp[:, hi * 256:(hi + 1) * 256],
                                           kT_s, b, h, c0, cosb, sinb)
                                rope_spill(qp[h // 2][:, (h % 2) * 256:
                                                      (h % 2) * 256 + 256],
                                           qT_s, b, h, c0, cosb, sinb)

            # ================= Phase B: attention =================
            if True:
                with tc.tile_pool(name="mkb", bufs=16) as pmk, \
                     tc.tile_pool(name="ktb", bufs=2) as pkt, \
                     tc.tile_pool(name="qtb", bufs=2) as pqt, \
                     tc.tile_pool(name="ptb", bufs=4) as ppt, \
                     tc.tile_pool(name="bsm", bufs=4) as pbs, \
                     tc.tile_pool(name="wob", bufs=1) as pwo, \
                     tc.tile_pool(name="bpsum", bufs=1, space="PSUM") as bps:

                    wo_t = {}
                    for b in range(B):
                        vt = [vt_tiles[(b, kt)] for kt in range(S // 128)]
                        mk = []
                        for qt in range(S // 128):
                            m_t = pmk.tile([128, 128], F32,
                                           name=f"m{b}_{qt}", tag="mk")
                            nc.gpsimd.dma_start(m_t[:], msk_d[b, qt])
                            mk.append(m_t)

                        for hl in range(GS):
                            kt_sb = pkt.tile([128, S], BF16, tag="kt")
                            nc.sync.dma_start(kt_sb[:], kT_s[b, hl])
                            for qc in range(NQC):
                                qt_sb = pqt.tile([128, 512], BF16, tag="qt")
                                nc.sync.dma_start(
                                    qt_sb[:],
                                    qT_s[b, hl, :, qc * 512:(qc + 1) * 512])
                                o_ps = [bps.tile([128, 512], F32, tag="ob",
                                                 bufs=4, name=f"ob{i}")
                                        for i in range(4)]
                                nkt = 4 * qc + 4
                                for kt in range(nkt):
                                    sc = bps.tile([128, 512], F32, tag="sc",
                                                  bufs=2)
                                    nc.tensor.matmul(
                                        sc[:],
                                        kt_sb[:, kt * 128:(kt + 1) * 128],
                                        qt_sb[:], start=True, stop=True)
                                    if kt >= 4 * qc:
                                        sub = kt - 4 * qc
                                        sl = sc[:, sub * 128:(sub + 1) * 128]
                                        nc.vector.tensor_add(sl, sl, mk[kt][:])
                                    pt = ppt.tile([128, 512], BF16, tag="pt")
                                    nc.scalar.activation(pt[:], sc[:], EXP)
                                    for sub in range(4):
                                        if kt <= 4 * qc + sub:
                                            nc.tensor.matmul(
                                                o_ps[sub][:, :FGD + 2],
                                                pt[:, sub * 128:
                                                   (sub + 1) * 128],
                                                vt[kt][:],
                                                start=(kt == 0),
                                                stop=(kt == 4 * qc + sub))
                                recip = pbs.tile([128, 4], F32, tag="rc")
                                for sub in range(4):
                                    nc.vector.reciprocal(
                                        recip[:, sub:sub + 1],
                                        o_ps[sub][:, FGD:FGD + 1])
                                for sub in range(4):
                                    o_sb = pbs.tile([128, FGD], BF16,
                                                    tag="osb")
                                    nc.vector.tensor_scalar_mul(
                                        o_sb[:], o_ps[sub][:, :FGD],
                                        recip[:, sub:sub + 1])
                                    tp2 = bps.tile([128, 256], BF16, tag="tp2",
                                                   bufs=2)
                                    for fp in range(2):
                                        nc.tensor.transpose(
                                            tp2[:, fp * 128:(fp + 1) * 128],
                                            o_sb[:, fp * 128:(fp + 1) * 128],
                                            ident[:])
                                    col = qc * 512 + sub * 128
                                    eng = nc.scalar if sub % 2 else nc.vector
                                    for fp in range(2):
                                        _copy(eng,
                                              ot[(b, hl, fp)][:, col:col + 128],
                                              tp2[:, fp * 128:(fp + 1) * 128])

                        if b == 0:
                            # prefetch Wo while b=1 attention runs
                            for half in range(2):
                                wo_t[half] = pwo.tile(
                                    [128, 8 * 2048], BF16,
                                    name=f"wo{half}", tag=f"wo{half}")
                                for j in range(4):
                                    for fp in range(2):
                                        nc.gpsimd.dma_start(
                                            wo_t[half][:, (j * 2 + fp) * 2048:
                                                 (j * 2 + fp + 1) * 2048],
                                            wo_d[j, fp, :,
                                                 half * 2048:(half + 1) * 2048])

                # ================= Phase C: output projection =================
                with tc.tile_pool(name="cev", bufs=4) as pcev, \
                     tc.tile_pool(name="cpsum", bufs=1, space="PSUM") as cps:
                    for half in range(2):
                        for b in range(B):
                            for tt in range(S // 128):
                                hl = tt // 4
                                for mci in range(4):
                                    mc = half * 4 + mci
                                    ops = cps.tile([128, 512], F32, tag="oc",
                                                   bufs=4)
                                    for j in range(4):
                                        for fp in range(2):
                                            otr = ot[(b, hl, fp)][:].rearrange(
                                                "p (x s) -> p x s", s=4)
                                            lhsT = otr[:, (tt % 4) * 128:
                                                       (tt % 4 + 1) * 128, j]
                                            rhs = wo_t[half][
                                                :, (j * 2 + fp) * 2048
                                                + mci * 512:
                                                (j * 2 + fp) * 2048
                                                + (mci + 1) * 512]
                                            nc.tensor.matmul(
                                                ops[:], lhsT, rhs,
                                                start=(j == 0 and fp == 0),
                                                stop=(j == 3 and fp == 1))
                                    ev = pcev.tile([128, 512], F32, tag="cev")
                                    eng = nc.scalar if (tt + mci) % 2 else nc.vector
                                    _copy(eng, ev[:], ops[:])
                                    nc.sync.dma_start(
                                        out_d[b, tt * 128:(tt + 1) * 128,
                                              mc * 512:(mc + 1) * 512],
                                        ev[:])

            _pvt_cm.__exit__(None, None, None)
            _potb_cm.__exit__(None, None, None)

    nc.compile()
    return nc


def _host_prep(inputs):
    """Slice/transposes per core; returns (in_maps, fallback_needed)."""
    hs = np.ascontiguousarray(inputs["hidden_states"], dtype=np.float32)
    mask = np.ascontiguousarray(inputs["attention_mask"], dtype=np.float32)
    pos = np.asarray(inputs["position_ids"])
    Wq = np.asarray(inputs["Wq"], dtype=np.float32)
    WVT = np.asarray(inputs["WVT"], dtype=np.float32)
    U = np.asarray(inputs["U"], dtype=np.float32)
    Wv = np.asarray(inputs["Wv"], dtype=np.float32)
    Wo = np.asarray(inputs["Wo"], dtype=np.float32)

    # Verify causal-family mask: strictly-lower 128-blocks all zero,
    # strictly-upper all <= -1e8 (else fall back to numpy reference).
    nt = S // 128
    mb = mask.reshape(B, nt, 128, nt, 128).transpose(0, 1, 3, 2, 4)
    lower_ok = True
    for b in range(B):
        for i in range(nt):
            for k in range(nt):
                blk = mb[b, i, k]
                if k < i and not np.all(blk == 0.0):
                    lower_ok = False
                if k > i and not np.all(blk <= -1e8):
                    lower_ok = False
    if not lower_ok:
        return None, True

    hidT = hs.transpose(0, 2, 1).astype(NPBF)               # (B, HID, S) bf16

    # RoPE tables, transposed (d on partitions); sin sign-folded for the
    # transposed rotate-half.
    inv = 1.0 / (THETA ** (np.arange(0, D, 2, dtype=np.float32) / D))
    fr = pos.astype(np.float32)[..., None] * inv                # (B, S, 64)
    emb = np.concatenate([fr, fr], axis=-1)                     # (B, S, 128)
    cosT = np.ascontiguousarray(
        np.cos(emb).astype(np.float32).transpose(0, 2, 1)).astype(NPBF)
    sinT = np.sin(emb).astype(np.float32).transpose(0, 2, 1)    # (B, 128, S)
    sinTf = np.concatenate([-sinT[:, :64], sinT[:, 64:]], axis=1)
    sinTf = np.ascontiguousarray(sinTf).astype(NPBF)

    # Transposed diagonal mask tiles (k, q)
    maskT = np.ascontiguousarray(
        np.stack([np.stack([mask[b, 0, t * 128:(t + 1) * 128,
                                 t * 128:(t + 1) * 128].T
                            for t in range(nt)]) for b in range(B)]))

    scale = np.float32(1.0 / np.sqrt(D))
    in_maps = []
    for g in range(NCORES):
        wqT = np.ascontiguousarray(
            Wq[g * GD:(g + 1) * GD, :].T * scale).astype(NPBF)
        wkT = np.ascontiguousarray(WVT[g * RK:(g + 1) * RK, :].T).astype(NPBF)
        wvT = np.ascontiguousarray(Wv[g * RK:(g + 1) * RK, :].T).astype(NPBF)
        Ug = U[:, g * RK:(g + 1) * RK]                           # (GD, RK)
        ugT = np.stack([np.ascontiguousarray(Ug[:, rt * 128:(rt + 1) * 128].T)
                        for rt in range(2)]).astype(NPBF)        # (2,128,GD)
        woT = np.empty((4, 2, 128, HID), np.float32)
        for j in range(4):
            base = j * 2048 + g * FGD
            blk = Wo[:, base:base + FGD].T                       # (256, 4096)
            woT[j, 0] = blk[:128]
            woT[j, 1] = blk[128:]
        in_maps.append(dict(hidT=hidT, wqT=wqT, wkT=wkT, wvT=wvT, ugT=ugT,
                            cosT=cosT, sinT=sinTf, maskT=maskT,
                            woT=woT.astype(NPBF)))
    return in_maps, False


def _numpy_fallback(inputs):
    hs = np.asarray(inputs["hidden_states"], np.float32)
    mask = np.asarray(inputs["attention_mask"], np.float32)
    pos = np.asarray(inputs["position_ids"])
    Wq, WVT, U, Wv, Wo = (np.asarray(inputs[k], np.float32)
                          for k in ["Wq", "WVT", "U", "Wv", "Wo"])
    b, q = hs.shape[:2]
    qs = (hs @ Wq.T).reshape(b, q, NH, D).transpose(0, 2, 1, 3)
    klat = (hs @ WVT.T).reshape(b, q, G, RK).transpose(0, 2, 1, 3)
    vlat = (hs @ Wv.T).reshape(b, q, G, FGD).transpose(0, 2, 1, 3)
    Ugr = U.reshape(GD, G, RK)
    keys = np.einsum("bgsr,dgr->bgsd", klat, Ugr)
    keys = keys.transpose(0, 2, 1, 3).reshape(b, q, NH, D).transpose(0, 2, 1, 3)
    inv = 1.0 / (THETA ** (np.arange(0, D, 2, dtype=np.float32) / D))
    fr = pos.astype(np.float32)[..., None] * inv
    emb = np.concatenate([fr, fr], -1)
    cos, sin = np.cos(emb)[:, None], np.sin(emb)[:, None]

    def rot(x):
        return np.concatenate([-x[..., D // 2:], x[..., :D // 2]], -1)
    qs = qs * cos + rot(qs) * sin
    keys = keys * cos + rot(keys) * sin
    att = np.einsum("bhqd,bhkd->bhqk", qs, keys) / np.sqrt(D).astype(np.float32)
    att = att + mask
    att = att - att.max(-1, keepdims=True)
    att = np.exp(att)
    att = att / att.sum(-1, keepdims=True)
    aw = att.reshape(b, G, q * GS, q)
    o = np.einsum("bgik,bgkf->bgif", aw.astype(np.float32),
                  vlat.astype(np.float32))
    o = o.transpose(0, 2, 1, 3).reshape(b, q, 8192)
    return (o @ Wo.T).astype(np.float32)


def _make_timing_fn(nc):
    """Build the sharded jit callable for this Bass module.

    Mirrors bass2jax.run_bass_via_pjrt's multi-core path; returns
    (fn, in_names, out_avals, sharding)."""
    import jax
    from jax.sharding import Mesh, NamedSharding, PartitionSpec
    from jax.experimental.shard_map import shard_map
    from concourse import bass2jax, mybir as _mb

    bass2jax.install_neuronx_cc_hook()

    part_name = (nc.partition_id_tensor.name
                 if nc.partition_id_tensor is not None else None)
    in_names, out_names, out_avals = [], [], []
    for alloc in nc.m.functions[0].allocations:
        if not isinstance(alloc, _mb.MemoryLocationSet):
            continue
        name = alloc.memorylocations[0].name
        if alloc.kind == "ExternalInput":
            if name != part_name:
                in_names.append(name)
        elif alloc.kind == "ExternalOutput":
            out_names.append(name)
            out_avals.append(jax.core.ShapedArray(
                tuple(alloc.tensor_shape), _mb.dt.np(alloc.dtype)))
    n_params = len(in_names)
    all_names = in_names + out_names
    if part_name is not None:
        all_names = all_names + [part_name]

    def _body(*args):
        operands = list(args)
        if part_name is not None:
            operands.append(bass2jax.partition_id_tensor())
        outs = bass2jax._bass_exec_p.bind(
            *operands,
            out_avals=tuple(out_avals),
            in_names=tuple(all_names),
            out_names=tuple(out_names),
            lowering_input_output_aliases=(),
            sim_require_finite=True,
            sim_require_nnan=True,
            nc=nc,
        )
        return tuple(outs)

    devices = jax.devices()[:NCORES]
    mesh = Mesh(np.asarray(devices), ("core",))
    spec = PartitionSpec("core")
    n_outs = len(out_names)
    fn = jax.jit(
        shard_map(_body, mesh=mesh, in_specs=(spec,) * (n_params + n_outs),
                  out_specs=(spec,) * n_outs, check_rep=False),
        keep_unused=True,
    )
    return fn, in_names, out_names, out_avals, NamedSharding(mesh, spec)


def _run_spmd(nc, in_maps, time_iters=0):
    """Execute the SPMD kernel on the first NCORES neuron devices via PJRT.

    Keeps inputs device-resident so repeated executions can be wall-clocked.
    Returns (results_per_core, exec_ns_best or None)."""
    import time as _time

    import jax

    if "timing_fn" not in _NC_CACHE:
        _NC_CACHE["timing_fn"] = _make_timing_fn(nc)
    fn, in_names, out_names, out_avals, sharding = _NC_CACHE["timing_fn"]
    dev_in = [
        jax.device_put(
            np.concatenate([np.asarray(m[name]) for m in in_maps], axis=0),
            sharding)
        for name in in_names
    ]
    dev_zero = [
        jax.device_put(
            np.zeros((NCORES * a.shape[0], *a.shape[1:]), a.dtype), sharding)
        for a in out_avals
    ]
    out = jax.block_until_ready(fn(*dev_in, *dev_zero))

    exec_ns = None
    if time_iters > 0:
        times = []
        for _ in range(time_iters):
            t0 = _time.perf_counter()
            r = jax.block_until_ready(fn(*dev_in, *dev_zero))
            times.append(_time.perf_counter() - t0)
        del r
        exec_ns = int(min(times) * 1e9)
        _NC_CACHE["bench_times"] = times

    results = []
    for c in range(NCORES):
        results.append({
            name: np.asarray(out[i]).reshape(NCORES, *out_avals[i].shape)[c]
            for i, name in enumerate(out_names)
        })
    return results, exec_ns


def kernel(**inputs):
    import os

    in_maps, fallback = _host_prep(inputs)
    if fallback:
        return _numpy_fallback(inputs)

    _install_loud_cc_hook()
    if "nc" not in _NC_CACHE:
        _NC_CACHE["nc"] = _build_nc()
    nc = _NC_CACHE["nc"]

    iters = int(os.environ.get("TRN_KERNEL_TIME_ITERS", "0"))
    results, exec_ns = _run_spmd(nc, in_maps, time_iters=iters)
    _NC_CACHE["last_exec_ns"] = exec_ns

    acc = np.zeros((B, S, HID), np.float64)
    for r in results:
        acc += r["out"].astype(np.float64)
    return acc.astype(np.float32)
